# revision 11
# baseline (speedup 1.0000x reference)
"""GNN message-passing (Convolve) kernel for Trainium2, 8 NeuronCores.

Reference computation (B=8, N=8192, C=256, H=256, O=256, K=64):
    g   = embeddings[:, neighbor_set, :]                     # [B, K, C]
    h   = leaky_relu(g @ Qw + Qb)                            # [B, K, H]
    w   = weights[neighbor_set, node_id]                     # [K]
    s   = sum_k h * w / (sum_k w + eps)                      # [B, H]
    z   = concat(embeddings[:, node_id, :], s)               # [B, C+H]
    o   = leaky_relu(z @ Ww + Wb)                            # [B, O]
    out = o / (||o||_2 + eps)                                # [B, O]

Sharding: data-parallel over the batch axis -- core b handles batch b.
The neighbor gather and all layout transforms are host-side (indices are
host-visible; the baseline already host-gathered the weight column).  The
normalized neighbor weights wn = w/(sum w + eps) are host-folded too.
All matmul operands ship as bf16 (fp8 was tried: relative errors on
zero-mean dot products don't average down, giving ~4% > the 2e-2 gate).

  big1 [128, 642] bf16  (one DMA, everything the h-chain needs):
     cols   0:128  gT interleaved:  big1[p, j*64+k] = emb[nbr[k], 2p+j]
     cols 128:130  node cols:       big1[p, 128+j]  = emb[node, 2p+j]
     cols 130:642  qw, contraction-interleaved, H cols permuted so h
                   lands in z-chunk order:
                   big1[p, 130+j2*256+m] = Qw[2p+j2, perm[m]],
                   perm = [0,2,...,254,1,3,...,255]
  big2 [128, 1024] bf16 -- Ww row-interleaved by 2:
     big2[p, j*256+o]     = Ww[2p+j, o]        (node-embedding half)
     big2[p, 512+j*256+o] = Ww[256+2p+j, o]    (s half)
  wnb  [64, 1] bf16 -- wn = w/(sum w + eps)  (host-computed)

Device dataflow (PSUM accumulates fp32):
    h_p[64,256] = sum_j gT_j.T @ qw_j ; h_l = Prelu(h_p) (bf16)
    x_p[1,256]  = sum_j node_j.T @ ww_node_j          (fills PE idle slot)
    s_j[128,1]  = h_l[:, j*128:+128].T @ wnb -> zs_j (bf16)
    x_p        += sum_j zs_j.T @ ww_s_j
    o = Prelu(x_p) ; n2 = accum(Square(o)) ; out = o * 1/sqrt(n2)
eps terms are dropped: den ~ U(0,1)-sum of 64 ~ 32, ||o|| >> eps.
ACT warms the 'sqrt_and_others' table (sqrt/prelu/square/copy) once.
"""

import functools

import numpy as np
import ml_dtypes

import concourse.bacc as bacc
import concourse.bass as bass
import concourse.mybir as mybir
import concourse.tile as tile
from concourse.bass_utils import run_bass_kernel_spmd

B, N, C, H, O, K = 8, 8192, 256, 256, 256, 64
ALPHA = 0.3
EPS = 1e-6
F32 = mybir.dt.float32
BF16 = mybir.dt.bfloat16
N_CORES = 8
MULT = mybir.AluOpType.mult
ADD = mybir.AluOpType.add
AF = mybir.ActivationFunctionType

BF = ml_dtypes.bfloat16
PERM = np.concatenate([np.arange(0, 256, 2), np.arange(1, 256, 2)])


def _build_program(has_qb: bool, has_wb: bool) -> bass.Bass:
    nc = bacc.Bacc(None, target_bir_lowering=False, debug=False)

    big1 = nc.dram_tensor("big1", [128, 642], BF16, kind="ExternalInput")
    big2 = nc.dram_tensor("big2", [128, 1024], BF16, kind="ExternalInput")
    wnb = nc.dram_tensor("wnb", [K, 1], BF16, kind="ExternalInput")
    if has_qb:
        qbd = nc.dram_tensor("qb", [1, H], BF16, kind="ExternalInput")
    if has_wb:
        wbd = nc.dram_tensor("wb", [1, O], F32, kind="ExternalInput")
    out_d = nc.dram_tensor("out", [1, O], F32, kind="ExternalOutput")

    with tile.TileContext(nc) as tc:
        with (
            tc.tile_pool(name="sb", bufs=1) as sb,
            tc.tile_pool(name="ps", bufs=1, space="PSUM") as ps,
        ):
            # ---- sync HWDGE: big1 (h-chain) first, then big2 ----
            gs = sb.tile([128, 642], BF16)
            nc.sync.dma_start(out=gs[:], in_=big1[:])
            ww = sb.tile([128, 1024], BF16)
            nc.sync.dma_start(out=ww[:], in_=big2[:])
            # ---- scalar HWDGE: tiny wn column, then ACT table warm ----
            wn = sb.tile([K, 1], BF16)
            nc.scalar.dma_start(out=wn[:], in_=wnb[:])
            if has_qb:
                qb_r = sb.tile([1, H], BF16)
                nc.scalar.dma_start(out=qb_r[:], in_=qbd[:])
            if has_wb:
                wb_r = sb.tile([1, O], F32)
                nc.scalar.dma_start(out=wb_r[:], in_=wbd[:])
            warm_in = sb.tile([1, 1], F32)
            nc.vector.memset(warm_in[:], 1.0)
            warm_out = sb.tile([1, 1], F32)
            nc.scalar.activation(out=warm_out[:], in_=warm_in[:], func=AF.Sqrt)
            if has_qb:
                ones_r = sb.tile([1, K], BF16)
                nc.vector.memset(ones_r[:], 1.0)

            # ---- h = Prelu(sum_j gT_j.T @ qw_j (+ Qb)) ----
            h_p = ps.tile([K, H], F32, tag="h")
            nc.tensor.matmul(
                out=h_p[:], lhsT=gs[:, 0:64], rhs=gs[:, 130:386],
                start=True, stop=False, skip_group_check=True,
            )
            nc.tensor.matmul(
                out=h_p[:], lhsT=gs[:, 64:128], rhs=gs[:, 386:642],
                start=False, stop=not has_qb, skip_group_check=True,
            )
            if has_qb:
                nc.tensor.matmul(
                    out=h_p[:], lhsT=ones_r[:], rhs=qb_r[:],
                    start=False, stop=True, skip_group_check=True,
                )
            h_l = sb.tile([K, H], BF16)
            nc.scalar.activation(out=h_l[:], in_=h_p[:], func=AF.Prelu, alpha=ALPHA)

            # ---- x group: node part fills the PE slot while ACT runs ----
            x_p = ps.tile([1, O], F32, tag="x")
            nc.tensor.matmul(
                out=x_p[:], lhsT=gs[:, 128:129], rhs=ww[:, 0:256],
                start=True, stop=False, skip_group_check=True,
            )
            nc.tensor.matmul(
                out=x_p[:], lhsT=gs[:, 129:130], rhs=ww[:, 256:512],
                start=False, stop=False, skip_group_check=True,
            )

            # ---- s cols (normalized via wn) -> bf16 z chunks ----
            s_ps = []
            for j in range(2):
                s_p = ps.tile([128, 1], F32, tag=f"s{j}")
                nc.tensor.matmul(
                    out=s_p[:], lhsT=h_l[:, 128 * j : 128 * (j + 1)], rhs=wn[:],
                    start=True, stop=True, skip_group_check=True,
                )
                s_ps.append(s_p)
            zs = []
            for j in range(2):
                z = sb.tile([128, 1], BF16)
                nc.vector.tensor_copy(out=z[:], in_=s_ps[j][:])
                zs.append(z)
            nc.tensor.matmul(
                out=x_p[:], lhsT=zs[0][:], rhs=ww[:, 512:768],
                start=False, stop=False, skip_group_check=True,
            )
            nc.tensor.matmul(
                out=x_p[:], lhsT=zs[1][:], rhs=ww[:, 768:1024],
                start=False, stop=True, skip_group_check=True,
            )

            # ---- o = Prelu(x (+Wb)); out = o / sqrt(sum o^2) ----
            if has_wb:
                x_s = sb.tile([1, O], F32)
                nc.vector.tensor_tensor(out=x_s[:], in0=x_p[:], in1=wb_r[:], op=ADD)
                act_src = x_s
            else:
                act_src = x_p
            o2 = sb.tile([1, O], F32)
            nc.scalar.activation(out=o2[:], in_=act_src[:], func=AF.Prelu, alpha=ALPHA)
            sq = sb.tile([1, O], F32)
            n2 = sb.tile([1, 1], F32)
            nc.scalar.activation(out=sq[:], in_=o2[:], func=AF.Square, accum_out=n2[:])
            nrm = sb.tile([1, 1], F32)
            nc.scalar.activation(out=nrm[:], in_=n2[:], func=AF.Sqrt)
            rec2 = sb.tile([1, 1], F32)
            nc.vector.reciprocal(rec2[:], nrm[:])
            res = sb.tile([1, O], F32)
            nc.vector.tensor_scalar_mul(res[:], o2[:], rec2[:])

            nc.sync.dma_start(out=out_d[:], in_=res[:], single_packet=True)

    nc.finalize()
    return nc


@functools.lru_cache(maxsize=4)
def _program(has_qb: bool, has_wb: bool) -> bass.Bass:
    return _build_program(has_qb, has_wb)


def kernel(
    embeddings: np.ndarray,
    weights: np.ndarray,
    Qw: np.ndarray,
    Qb: np.ndarray,
    Ww: np.ndarray,
    Wb: np.ndarray,
    neighbor_set: np.ndarray,
    node_id,
    _trace: bool = False,
):
    node_id = int(np.asarray(node_id))
    nbr = np.asarray(neighbor_set).astype(np.int64).reshape(K)
    has_qb = bool(np.any(Qb))
    has_wb = bool(np.any(Wb))

    # qw block: [p, j2*256 + m] = Qw[2p+j2, PERM[m]]
    qw_t = np.asarray(Qw, dtype=np.float32)[:, PERM].reshape(128, 512).astype(BF)
    # ww: rows interleaved by 2 within each 256-row half
    ww_f = np.asarray(Ww, dtype=np.float32)
    big2 = np.concatenate(
        [ww_f[0:256].reshape(128, 512), ww_f[256:512].reshape(128, 512)], axis=1
    ).astype(BF)

    w_col = np.asarray(weights[nbr, node_id], dtype=np.float32)
    wnb = (w_col / (w_col.sum() + EPS)).astype(BF).reshape(K, 1)

    nc = _program(has_qb, has_wb)
    in_maps = []
    for b in range(N_CORES):
        emb = np.asarray(embeddings[b], dtype=np.float32)
        g = emb[nbr]  # [K, C]
        big1 = np.empty((128, 642), dtype=BF)
        big1[:, 0:128] = g.T.reshape(128, 128)  # [p, j*64+k] = g[k, 2p+j]
        big1[:, 128:130] = emb[node_id].reshape(128, 2)
        big1[:, 130:642] = qw_t
        m = {"big1": big1, "big2": big2, "wnb": wnb}
        if has_qb:
            m["qb"] = np.asarray(Qb, dtype=np.float32)[PERM].reshape(1, H).astype(BF)
        if has_wb:
            m["wb"] = np.asarray(Wb, dtype=np.float32).reshape(1, O)
        in_maps.append(m)
    r = run_bass_kernel_spmd(nc, in_maps, list(range(N_CORES)), trace=_trace)
    out = np.stack([r.results[b]["out"][0] for b in range(N_CORES)], axis=0)
    if _trace:
        return out, r
    return out


# revision 12
# speedup vs baseline: 1.0554x; 1.0554x over previous
"""GNN message-passing (Convolve) kernel for Trainium2, 8 NeuronCores.

Reference computation (B=8, N=8192, C=256, H=256, O=256, K=64):
    g   = embeddings[:, neighbor_set, :]                     # [B, K, C]
    h   = leaky_relu(g @ Qw + Qb)                            # [B, K, H]
    w   = weights[neighbor_set, node_id]                     # [K]
    s   = sum_k h * w / (sum_k w + eps)                      # [B, H]
    z   = concat(embeddings[:, node_id, :], s)               # [B, C+H]
    o   = leaky_relu(z @ Ww + Wb)                            # [B, O]
    out = o / (||o||_2 + eps)                                # [B, O]

Sharding: data-parallel over the batch axis -- core b handles batch b.
The neighbor gather and all layout transforms are host-side (indices are
host-visible; the baseline already host-gathered the weight column).  The
normalized neighbor weights wn = w/(sum w + eps) are host-folded too.
All matmul operands ship as bf16 (fp8 was tried: relative errors on
zero-mean dot products don't average down, giving ~4% > the 2e-2 gate).

The kernel is pure latency: ~7us fixed engine-start preamble, then a
serial DMA -> PE -> ACT -> PE -> DVE -> PE -> ACT/DVE -> DMA chain.
DMAs are split across the two HWDGE engines so their 16-ring queues
stream in parallel:
  sync   gc [128, 130]: cols 0:128 gT interleaved (gc[p, j*64+k] =
             emb[nbr[k], 2p+j]); cols 128:130 node cols emb[node, 2p+j]
         ww [128, 1024]: Ww row-interleaved by 2 per 256-row half
             (ww[p, j*256+o] = Ww[2p+j, o]; +512 for the s half)
  scalar qw [128, 512]: contraction-interleaved, H cols permuted so h
             lands in z-chunk order (qw[p, j2*256+m] = Qw[2p+j2, perm[m]],
             perm = evens then odds)
         wnb [64, 1] bf16: wn = w/(sum w + eps)

Device dataflow (PSUM accumulates fp32):
    h_p[64,256] = sum_j gT_j.T @ qw_j ; h_l = Prelu(h_p) (bf16)
    x_p[1,256]  = sum_j node_j.T @ ww_node_j          (fills PE idle slot)
    s_pp[128,2]: col j = h_l[:, j*128:+128].T @ wnb ; one cast -> zs bf16
    x_p        += sum_j zs[:, j].T @ ww_s_j
    o = Prelu(x_p) ; n2 = accum(Square(o)) ; out = o * 1/sqrt(n2)
eps terms are dropped: den ~ U(0,1)-sum of 64 ~ 32, ||o|| >> eps.
ACT warms Sqrt+Copy so one 'sqrt_and_others' table load covers
sqrt/prelu/square/copy before real data arrives.
"""

import functools

import numpy as np
import ml_dtypes

import concourse.bacc as bacc
import concourse.bass as bass
import concourse.mybir as mybir
import concourse.tile as tile
from concourse.bass_utils import run_bass_kernel_spmd

B, N, C, H, O, K = 8, 8192, 256, 256, 256, 64
ALPHA = 0.3
EPS = 1e-6
F32 = mybir.dt.float32
BF16 = mybir.dt.bfloat16
N_CORES = 8
MULT = mybir.AluOpType.mult
ADD = mybir.AluOpType.add
AF = mybir.ActivationFunctionType

BF = ml_dtypes.bfloat16
PERM = np.concatenate([np.arange(0, 256, 2), np.arange(1, 256, 2)])


def _build_program(has_qb: bool, has_wb: bool) -> bass.Bass:
    nc = bacc.Bacc(None, target_bir_lowering=False, debug=False)

    gcd = nc.dram_tensor("gc", [128, 130], BF16, kind="ExternalInput")
    wwd = nc.dram_tensor("ww", [128, 1024], BF16, kind="ExternalInput")
    qwd = nc.dram_tensor("qw", [128, 512], BF16, kind="ExternalInput")
    wnd = nc.dram_tensor("wnb", [K, 1], BF16, kind="ExternalInput")
    if has_qb:
        qbd = nc.dram_tensor("qb", [1, H], BF16, kind="ExternalInput")
    if has_wb:
        wbd = nc.dram_tensor("wb", [1, O], F32, kind="ExternalInput")
    out_d = nc.dram_tensor("out", [1, O], F32, kind="ExternalOutput")

    with tile.TileContext(nc) as tc:
        with (
            tc.tile_pool(name="sb", bufs=1) as sb,
            tc.tile_pool(name="ps", bufs=1, space="PSUM") as ps,
        ):
            # ---- sync HWDGE rings: tiny gather block first, then Ww ----
            gc = sb.tile([128, 130], BF16)
            nc.sync.dma_start(out=gc[:], in_=gcd[:])
            ww = sb.tile([128, 1024], BF16)
            nc.sync.dma_start(out=ww[:], in_=wwd[:])
            # ---- scalar HWDGE rings (parallel): Qw, then wn column ----
            qw = sb.tile([128, 512], BF16)
            nc.scalar.dma_start(out=qw[:], in_=qwd[:])
            wn = sb.tile([K, 1], BF16)
            nc.scalar.dma_start(out=wn[:], in_=wnd[:])
            if has_qb:
                qb_r = sb.tile([1, H], BF16)
                nc.scalar.dma_start(out=qb_r[:], in_=qbd[:])
            if has_wb:
                wb_r = sb.tile([1, O], F32)
                nc.scalar.dma_start(out=wb_r[:], in_=wbd[:])
            # ---- ACT table warm (Sqrt+Copy => one sqrt_and_others load) ----
            warm_in = sb.tile([1, 1], F32)
            nc.vector.memset(warm_in[:], 1.0)
            warm_out = sb.tile([1, 1], F32)
            nc.scalar.activation(out=warm_out[:], in_=warm_in[:], func=AF.Sqrt)
            warm_out2 = sb.tile([1, 1], F32)
            nc.scalar.activation(out=warm_out2[:], in_=warm_in[:], func=AF.Copy)
            if has_qb:
                ones_r = sb.tile([1, K], BF16)
                nc.vector.memset(ones_r[:], 1.0)

            # ---- h = Prelu(sum_j gT_j.T @ qw_j (+ Qb)) ----
            h_p = ps.tile([K, H], F32, tag="h")
            nc.tensor.matmul(
                out=h_p[:], lhsT=gc[:, 0:64], rhs=qw[:, 0:256],
                start=True, stop=False, skip_group_check=True,
            )
            nc.tensor.matmul(
                out=h_p[:], lhsT=gc[:, 64:128], rhs=qw[:, 256:512],
                start=False, stop=not has_qb, skip_group_check=True,
            )
            if has_qb:
                nc.tensor.matmul(
                    out=h_p[:], lhsT=ones_r[:], rhs=qb_r[:],
                    start=False, stop=True, skip_group_check=True,
                )
            h_l = sb.tile([K, H], BF16)
            nc.scalar.activation(out=h_l[:], in_=h_p[:], func=AF.Prelu, alpha=ALPHA)

            # ---- x group: node part fills the PE slot while ACT runs ----
            x_p = ps.tile([1, O], F32, tag="x")
            nc.tensor.matmul(
                out=x_p[:], lhsT=gc[:, 128:129], rhs=ww[:, 0:256],
                start=True, stop=False, skip_group_check=True,
            )
            nc.tensor.matmul(
                out=x_p[:], lhsT=gc[:, 129:130], rhs=ww[:, 256:512],
                start=False, stop=False, skip_group_check=True,
            )

            # ---- s cols (normalized via wn): one PSUM tile, one cast ----
            s_pp = ps.tile([128, 2], F32, tag="s")
            for j in range(2):
                nc.tensor.matmul(
                    out=s_pp[:, j : j + 1],
                    lhsT=h_l[:, 128 * j : 128 * (j + 1)], rhs=wn[:],
                    start=True, stop=True, skip_group_check=True,
                )
            zs = sb.tile([128, 2], BF16)
            nc.vector.tensor_copy(out=zs[:], in_=s_pp[:])
            nc.tensor.matmul(
                out=x_p[:], lhsT=zs[:, 0:1], rhs=ww[:, 512:768],
                start=False, stop=False, skip_group_check=True,
            )
            nc.tensor.matmul(
                out=x_p[:], lhsT=zs[:, 1:2], rhs=ww[:, 768:1024],
                start=False, stop=True, skip_group_check=True,
            )

            # ---- o = Prelu(x (+Wb)); out = o / sqrt(sum o^2) ----
            if has_wb:
                x_s = sb.tile([1, O], F32)
                nc.vector.tensor_tensor(out=x_s[:], in0=x_p[:], in1=wb_r[:], op=ADD)
                act_src = x_s
            else:
                act_src = x_p
            o2 = sb.tile([1, O], F32)
            nc.scalar.activation(out=o2[:], in_=act_src[:], func=AF.Prelu, alpha=ALPHA)
            sq = sb.tile([1, O], F32)
            n2 = sb.tile([1, 1], F32)
            nc.scalar.activation(out=sq[:], in_=o2[:], func=AF.Square, accum_out=n2[:])
            nrm = sb.tile([1, 1], F32)
            nc.scalar.activation(out=nrm[:], in_=n2[:], func=AF.Sqrt)
            rec2 = sb.tile([1, 1], F32)
            nc.vector.reciprocal(rec2[:], nrm[:])
            res = sb.tile([1, O], F32)
            nc.vector.tensor_scalar_mul(res[:], o2[:], rec2[:])

            nc.sync.dma_start(out=out_d[:], in_=res[:])

    nc.finalize()
    return nc


@functools.lru_cache(maxsize=4)
def _program(has_qb: bool, has_wb: bool) -> bass.Bass:
    return _build_program(has_qb, has_wb)


def kernel(
    embeddings: np.ndarray,
    weights: np.ndarray,
    Qw: np.ndarray,
    Qb: np.ndarray,
    Ww: np.ndarray,
    Wb: np.ndarray,
    neighbor_set: np.ndarray,
    node_id,
    _trace: bool = False,
):
    node_id = int(np.asarray(node_id))
    nbr = np.asarray(neighbor_set).astype(np.int64).reshape(K)
    has_qb = bool(np.any(Qb))
    has_wb = bool(np.any(Wb))

    # qw block: [p, j2*256 + m] = Qw[2p+j2, PERM[m]]
    qw_t = np.asarray(Qw, dtype=np.float32)[:, PERM].reshape(128, 512).astype(BF)
    # ww: rows interleaved by 2 within each 256-row half
    ww_f = np.asarray(Ww, dtype=np.float32)
    ww_t = np.concatenate(
        [ww_f[0:256].reshape(128, 512), ww_f[256:512].reshape(128, 512)], axis=1
    ).astype(BF)

    w_col = np.asarray(weights[nbr, node_id], dtype=np.float32)
    wnb = (w_col / (w_col.sum() + EPS)).astype(BF).reshape(K, 1)

    nc = _program(has_qb, has_wb)
    in_maps = []
    for b in range(N_CORES):
        emb = np.asarray(embeddings[b], dtype=np.float32)
        g = emb[nbr]  # [K, C]
        gc = np.empty((128, 130), dtype=BF)
        gc[:, 0:128] = g.T.reshape(128, 128)  # [p, j*64+k] = g[k, 2p+j]
        gc[:, 128:130] = emb[node_id].reshape(128, 2)
        m = {"gc": gc, "ww": ww_t, "qw": qw_t, "wnb": wnb}
        if has_qb:
            m["qb"] = np.asarray(Qb, dtype=np.float32)[PERM].reshape(1, H).astype(BF)
        if has_wb:
            m["wb"] = np.asarray(Wb, dtype=np.float32).reshape(1, O)
        in_maps.append(m)
    r = run_bass_kernel_spmd(nc, in_maps, list(range(N_CORES)), trace=_trace)
    out = np.stack([r.results[b]["out"][0] for b in range(N_CORES)], axis=0)
    if _trace:
        return out, r
    return out
